# revision 33
# baseline (speedup 1.0000x reference)
"""Trainium2 Bass kernel for windowless 3D relative-position attention.

Full-input contract: kernel(**inputs) takes the unsharded numpy inputs and
returns the full [4, 2048, 256] output. Internally shards across 8 NeuronCores
as (batch b = core//2) x (head-group g = core%2, 4 heads each).

v3 design (ACT-exp is the pacemaker at ~1.03us per FD=1024 instruction; the
PE is assumed cold at 1.2 GHz due to HAM so per-slot PE work must fit under
the ACT slot):
  - TWO PASSES over all (quarter, m-tile) blocks, 2 heads per pass. Each
    tile is one [128, 2, 512] fp32 sc PSUM tile (2 banks), ONE exp
    (FD=1024), ONE DVE bias-mul (FD=1024, exp(bias) broadcast via stride-0
    AP), and 2 AV matmuls (2-way column-concurrent).
  - oa accumulator = ONE bank per (pass, quarter): [0:32]=num_h even,
    [32:64]=den, [64:96]=num_h odd, [96:128]=den. The oa/scratch ring
    (1 tag, bufs=2) is DOUBLE-buffered and also serves qk-proj and v-proj
    scratch, so quarter boundaries never stall: next block's AVs write the
    other bank while normalize reads this one.
  - PSUM = sc ring (3 bufs x 2 banks) + ring (2 x 1 bank) = 8 banks.
  - AV lhsT is [128 keys, 64]: cols 0:32 = v, 32:64 = ones -> the softmax
    denominator comes out REPLICATED on 32 psum partitions; normalization is
    reciprocal_approx_fast per bank + one fp32 mul per head (num stays in
    PSUM: the walrus verifier requires all SBUF inputs of a tensor_tensor
    to share a start partition, PSUM inputs are exempt).
  - boundary tasks (recip, nmuls; on pass 1 also the quarter's
    out-projection into a cycled sc-ring tile + copies + batched output
    DMA on the sync queue) are popped one per m-slot of the next block.
  - next block's first scores are issued before the trailing AVs (carried
    across the boundary in av_back) so ACT never gaps.
  - dummy exp activation issued at t~0 so the ~2.7us ACT table load hides
    under the input DMAs; prologue interleaves qk-chunks, v-proj groups
    and the first 4 tiles' scores/exp so ACT starts ~15us in.

The bias add is factored through the exponential: exp(s+bias) =
exp(s)*exp(bias), with exp(bias.T - C_SHIFT) precomputed on host in fp16
(C_SHIFT keeps products in fp16 range; it cancels in the softmax ratio).

Hardware constraints learned (do not re-attempt blindly): matmul PSUM out
is limited to one 2KB bank (512 fp32 cols); engine APs must start on
partition 0/32/64/96 and fit the quadrant; DVE ops cannot shift partitions
(two SBUF inputs must share a start partition); DMA cannot touch PSUM; DMA
partition stride must be 1 (stride-0 broadcast only from DRAM); tile pools
release LIFO; tile() with name= but no tag= makes the name the tag (one
ring per name!); the Tile scheduler reorders by sim-readiness+priority, so
in-order engine queues stall on mispredicted DMA arrivals; fp8 DoubleRow
gives no column-rate gain at K<128; GPSIMD DGE adds latency in
dependency-critical chains; HAM keeps PE at 1.2 GHz unless continuously
busy >=3.4us (ACT-gated cadence never re-warms it).
"""

import os
import sys
from contextlib import ExitStack

import numpy as np

sys.path.insert(0, "/opt/trn_rl_repo")

import concourse.bass as bass
import concourse.bacc as bacc
import concourse.tile as tile
from concourse import mybir
from concourse.bass_utils import run_bass_kernel_spmd

# Problem constants (hardcoded per contract)
B = 4
N = 2048
INP = 256
OUP = 256
HEADS = 8
DIM_HEAD = 32
SCALE = DIM_HEAD ** -0.5
HL = 4            # heads per core
MT = N // 128     # 16 m-tiles (keys)
NQ = 4            # 512-wide n (query) quarters
NQW = 512
C_SHIFT = 4.0

f32 = mybir.dt.float32
f16 = mybir.dt.float16

_LAST = {"exec_time_ns": None}


def _build_nc():
    nc = bacc.Bacc("TRN2", target_bir_lowering=False, debug=False)
    xT_d = nc.dram_tensor("xT", [2, 128, N], f16, kind="ExternalInput")
    wqk_d = nc.dram_tensor("w_qk", [2, 128, 256], f16, kind="ExternalInput")
    wv_d = nc.dram_tensor("w_v", [2, 128, 128], f16, kind="ExternalInput")
    wout_d = nc.dram_tensor("w_out2", [128, 256], f16, kind="ExternalInput")
    ebt_d = nc.dram_tensor("expbt", [N, N], f16, kind="ExternalInput")
    out_d = nc.dram_tensor("partial", [N, OUP], f32, kind="ExternalOutput")

    with ExitStack() as ctx:
        tc = ctx.enter_context(tile.TileContext(nc))
        consts = ctx.enter_context(tc.tile_pool(name="consts", bufs=1))

        ebt = consts.tile([128, MT, N], f16)          # [m%128, mtile, n]
        xT = consts.tile([128, 2, N], f16)
        wqk = consts.tile([128, 2, 256], f16)
        wv = consts.tile([128, 2, 128], f16)
        woutd = consts.tile([128, 256], f16)
        qkT = consts.tile([128, 2, N], f16)           # [:,0,:]=qT  [:,1,:]=kT
        vsb = consts.tile([128, MT, HL, 64], f16)     # [key, mtile, head, v|ones]
        aoutT = consts.tile([128, N], f16)            # [(h,d), n] normalized
        dummy = consts.tile([128, 8], f32)

        # dummy exp right away: the ~2.7us ACT table load hides under DMAs
        nc.vector.memset(dummy[:], 1.0)
        nc.scalar.activation(
            out=dummy[:, 0:4], in_=dummy[:, 4:8],
            func=mybir.ActivationFunctionType.Exp, scale=1.0,
        )

        # DMA order: x cols 0:1024 (covers qk-proj ch0+ch1) + w_qk first,
        # then x second halves, wv, then the ebt stream
        for kk in range(2):
            nc.sync.dma_start(out=xT[:, kk, 0:N // 2], in_=xT_d[kk, :, 0:N // 2])
        for kk in range(2):
            nc.sync.dma_start(out=wqk[:, kk, :], in_=wqk_d[kk])
        for kk in range(2):
            nc.sync.dma_start(out=xT[:, kk, N // 2:N], in_=xT_d[kk, :, N // 2:N])
        for kk in range(2):
            nc.sync.dma_start(out=wv[:, kk, :], in_=wv_d[kk])
        for m in range(2):
            nc.sync.dma_start(out=ebt[:, m, :], in_=ebt_d[m * 128:(m + 1) * 128, :])
        nc.sync.dma_start(out=woutd[:], in_=wout_d[:])
        for m in range(2, MT):
            nc.sync.dma_start(out=ebt[:, m, :], in_=ebt_d[m * 128:(m + 1) * 128, :])
        nc.gpsimd.memset(vsb[:], 1.0)

        with tc.tile_pool(name="awp", bufs=4) as awp, \
             tc.tile_pool(name="aw2p", bufs=6) as aw2p, \
             tc.tile_pool(name="otp", bufs=2) as otp, \
             tc.tile_pool(name="recp", bufs=2) as recp:
          with tc.tile_pool(name="sps", bufs=3, space="PSUM") as sps, \
               tc.tile_pool(name="pps", bufs=2, space="PSUM") as pps:

            def ring_tile(name):
                return pps.tile([128, NQW], f32, tag="ring", name=name)

            def issue_scores(p, m, ncol0):
                sc = sps.tile([128, 2, NQW], f32, tag="sc",
                              name=f"sc{p}_{m}")
                for hi in range(2):
                    hl = 2 * p + hi
                    nc.tensor.matmul(
                        sc[:, hi, :],
                        lhsT=qkT[32 * hl:32 * (hl + 1), 1,
                                 m * 128:(m + 1) * 128],
                        rhs=qkT[32 * hl:32 * (hl + 1), 0,
                                ncol0:ncol0 + NQW],
                        start=True, stop=True,
                        tile_position=(32 * hl, 0),
                    )
                return sc

            def issue_act_mul(p, m, ncol0, sc):
                aw = awp.tile([128, 2, NQW], f16, tag="aw")
                nc.scalar.activation(
                    out=aw[:], in_=sc[:],
                    func=mybir.ActivationFunctionType.Exp,
                    scale=SCALE,
                )
                ebs = ebt[:, m, ncol0:ncol0 + NQW]
                eb_b = bass.AP(
                    tensor=ebs.tensor, offset=ebs.offset,
                    ap=[ebs.ap[0], [0, 2], ebs.ap[1]],
                )
                aw2 = aw2p.tile([128, 2, NQW], f16, tag="aw2")
                nc.vector.tensor_mul(aw2[:], aw[:], eb_b)
                return aw2

            def issue_av(p, m, aw2, oa):
                for hi in range(2):
                    hl = 2 * p + hi
                    nc.tensor.matmul(
                        oa[64 * hi:64 * hi + 64, :],
                        lhsT=vsb[:, m, hl, :],
                        rhs=aw2[:, hi, :],
                        start=(m == 0), stop=(m == MT - 1),
                    )

            def make_boundary_tasks(p, qq, oa_q, tail=False):
                """Normalize (+ out-project on pass 1) block (p, qq); one
                task per m-slot of the next block. The out-projection goes
                into a freshly cycled sc-ring tile (2 banks hold all four
                [128, 256] n-tile outputs)."""
                rec = recp.tile([128, NQW], f32, tag="rec",
                                name=f"rec{p}_{qq}")
                tasks = []

                def recip():
                    nc.vector.reciprocal_approx_fast(out=rec[:], in_=oa_q[:])
                tasks.append(recip)

                def nmuls():
                    for hi in range(2):
                        hl = 2 * p + hi
                        po = 64 * hi
                        nc.vector.tensor_mul(
                            aoutT[32 * hl:32 * hl + 32,
                                  qq * NQW:(qq + 1) * NQW],
                            oa_q[po:po + 32, :],
                            rec[po + 32:po + 64, :],
                        )
                tasks.append(nmuls)

                if p == 1:
                    ot = otp.tile([128, 4, OUP], f32, tag="ot",
                                  name=f"ot{qq}")
                    prj = [None]

                    def make_prjmm(jp):
                        def prjmm():
                            if prj[0] is None:
                                prj[0] = sps.tile([128, 2, NQW], f32,
                                                  tag="sc", name=f"prj{qq}")
                            for j in (2 * jp, 2 * jp + 1):
                                nt = 4 * qq + j
                                pp = prj[0][:, j // 2,
                                            (j % 2) * OUP:(j % 2 + 1) * OUP]
                                nc.tensor.matmul(
                                    pp,
                                    lhsT=aoutT[:, nt * 128:(nt + 1) * 128],
                                    rhs=woutd[:],
                                    start=True, stop=True,
                                )
                        return prjmm

                    def make_pcopy(jp):
                        def pcopy():
                            for j in (2 * jp, 2 * jp + 1):
                                src = prj[0][:, j // 2,
                                             (j % 2) * OUP:(j % 2 + 1) * OUP]
                                if tail and j % 2 == 0:
                                    nc.scalar.copy(out=ot[:, j, :], in_=src)
                                else:
                                    nc.vector.tensor_copy(out=ot[:, j, :],
                                                          in_=src)
                            if jp == 1:
                                od = out_d[qq * 512:(qq + 1) * 512, :]
                                od4 = bass.AP(
                                    tensor=od.tensor, offset=od.offset,
                                    ap=[[OUP, 128], [OUP * 128, 4],
                                        [1, OUP]],
                                )
                                nc.sync.dma_start(out=od4, in_=ot[:])
                        return pcopy

                    tasks += [make_prjmm(0), make_prjmm(1),
                              make_pcopy(0), make_pcopy(1)]
                return tasks

            # ---- prologue: qk-proj chunks, first scores/exps, v-proj ----
            pro_scs = []      # scores tiles (pass 0, q0) t0..t4
            pro_aw2 = []      # aw2 tiles t0..t3

            def qk_chunk(ch):
                for mb in range(2):   # 0 -> q block, 1 -> k block
                    ps = ring_tile(f"qkps{ch}_{mb}")
                    for kk in range(2):
                        nc.tensor.matmul(
                            ps[:],
                            lhsT=wqk[:, kk, mb * 128:(mb + 1) * 128],
                            rhs=xT[:, kk, ch * 512:(ch + 1) * 512],
                            start=(kk == 0), stop=(kk == 1),
                        )
                    nc.vector.tensor_copy(
                        out=qkT[:, mb, ch * 512:(ch + 1) * 512], in_=ps[:]
                    )

            def pro_tile(t):
                if t >= 1:
                    pro_aw2.append(issue_act_mul(0, t - 1, 0, pro_scs[t - 1]))
                pro_scs.append(issue_scores(0, t, 0))

            def v_group(g):
                ps = ring_tile(f"vps{g}")
                for j in range(4):
                    nt = 4 * g + j
                    for kk in range(2):
                        nc.tensor.matmul(
                            ps[:, j * 128:(j + 1) * 128],
                            lhsT=xT[:, kk, nt * 128:(nt + 1) * 128],
                            rhs=wv[:, kk, :],
                            start=(kk == 0), stop=(kk == 1),
                        )
                ps4 = bass.AP(
                    tensor=ps.tensor, offset=ps.offset,
                    ap=[ps.ap[0], [128, 4], [32, 4], [1, 32]],
                )
                nc.vector.tensor_copy(out=vsb[:, 4 * g:4 * g + 4, :, 0:32],
                                      in_=ps4)

            qk_chunk(0)
            pro_tile(0)
            pro_tile(1)
            qk_chunk(1)
            pro_tile(2)
            qk_chunk(2)
            qk_chunk(3)
            pro_tile(3)
            v_group(0)
            pro_tile(4)
            v_group(1)
            v_group(2)
            v_group(3)

            # ---- main loop: 8 blocks of 16 m-slots ----
            tasks = []
            av_back = []
            next_scs = None
            for p in range(2):
                for q in range(NQ):
                    ncol0 = q * NQW
                    oa = ring_tile(f"oa{p}_{q}")

                    first = (p == 0 and q == 0)
                    if first:
                        scs = pro_scs[4]
                        av_back = [(0, t, pro_aw2[t], oa) for t in range(4)]
                        m0 = 4
                    else:
                        scs = next_scs
                        m0 = 0
                    for m in range(m0, MT):
                        aw2 = issue_act_mul(p, m, ncol0, scs)
                        if m + 1 < MT:
                            scs = issue_scores(p, m + 1, ncol0)
                        elif not (p == 1 and q == NQ - 1):
                            nq = (q + 1) % NQ
                            next_scs = issue_scores(p + (1 if nq == 0 else 0),
                                                    0, nq * NQW)
                        if not first and m == 0:
                            # previous block's carried AVs must be issued
                            # before task 0 (recip) reads the old oa bank
                            while av_back:
                                issue_av(*av_back.pop(0))
                        if tasks:
                            tasks.pop(0)()
                        av_back.append((p, m, aw2, oa))
                        if len(av_back) > 1:
                            issue_av(*av_back.pop(0))
                        if len(av_back) > 2 and m % 2 == 0:
                            issue_av(*av_back.pop(0))
                    tasks = make_boundary_tasks(p, q, oa,
                                                tail=(p == 1 and q == NQ - 1))

            # tail flush
            while av_back:
                issue_av(*av_back.pop(0))
            for t in tasks:
                t()
            tasks = []
    nc.compile()
    return nc


_NC_CACHE = {}


def kernel(x, w_qkv, bias_table, w_out, b_out, relative_pos):
    x = np.asarray(x, np.float32)
    w_qkv = np.asarray(w_qkv, np.float32)
    bias_table = np.asarray(bias_table, np.float32)
    w_out = np.asarray(w_out, np.float32)
    b_out = np.asarray(b_out, np.float32)
    relative_pos = np.asarray(relative_pos, np.int32)

    bias = bias_table[relative_pos, 0]                       # [n, m]
    expBT = np.exp(bias.T - C_SHIFT).astype(np.float16)      # [m, n]
    expBT = np.ascontiguousarray(expBT)

    if "nc" not in _NC_CACHE:
        _NC_CACHE["nc"] = _build_nc()
    nc = _NC_CACHE["nc"]

    in_maps = []
    for c in range(8):
        b, g = c // 2, c % 2
        w_qk = np.concatenate(
            [w_qkv[:, g * 128:(g + 1) * 128],
             w_qkv[:, 256 + g * 128:256 + (g + 1) * 128]], axis=1)
        in_maps.append({
            "xT": np.ascontiguousarray(x[b].T).reshape(2, 128, N).astype(np.float16),
            "w_qk": np.ascontiguousarray(w_qk).reshape(2, 128, 256).astype(np.float16),
            "w_v": np.ascontiguousarray(
                w_qkv[:, 512 + g * 128:512 + (g + 1) * 128]
            ).reshape(2, 128, 128).astype(np.float16),
            "w_out2": np.ascontiguousarray(
                w_out[g * 128:(g + 1) * 128, :]
            ).astype(np.float16),
            "expbt": expBT,
        })

    trace = bool(os.environ.get("KERNEL_TRACE"))
    res = run_bass_kernel_spmd(nc, in_maps, list(range(8)), trace=trace)
    _LAST["exec_time_ns"] = res.exec_time_ns
    _LAST["results"] = res

    parts = [np.asarray(res.results[c]["partial"], np.float32) for c in range(8)]
    out = np.stack([parts[2 * b] + parts[2 * b + 1] + b_out for b in range(B)])
    return out.astype(np.float32)
